# revision 59
# baseline (speedup 1.0000x reference)
"""Trainium2 Bass kernel for softmax RGB blend (pytorch3d NoLightShader).

Full inputs (N=8, H=512, W=512, K=8) are sharded batch-wise across 8
NeuronCores (one image per core); the blend is per-pixel so no cross-core
communication is needed.

Host-side input repack (per core, pure layout/dtype transforms):
  - mask folded into dists:  d' = masked ? +30000 : d   (fp16)
      (tanh(d'*5000) = 1  ->  q = 1, p = 0, exactly the masked case)
  - mask folded into z via quantization:
      zq = round((ZFAR - z)/D * 32767) * mask   (int16; 0 when masked,
      matching the reference's masked z_inv == 0 exactly)
  - colors fp16, k-major planar per phase:  [P, n, 3, K, T]
  - zq/d' k-major per phase: [P, n, K, T]
Output is written planar [P, n, 4, T] (r,g,b,a planes) and untransposed
on the host.

Math per pixel (K faces); everything is scaled by 256 (folded into pm2
and the delta bias) so that rcp = 1/denom' stays in fp16 range:
    th_k    = tanh(d_k*5000)          (ACT; masked -> 1)
    qq2_k   = 1 + th_k    (= 2q)      (ACT Copy;  prodq = prod_k qq2_k)
    alpha   = 1 - prodq/256           (ACT Copy, into out plane 3)
    zqmax   = max_k zq_k  (int16 TT-max tree -> f16; f16 rounding of
              zqmax is common-mode across k and cancels in the blend)
    zd_k    = zq_k - zqmax            (int16 - f16 bcast -> f16, 2x)
    ex_k    = exp(zd_k * S2)          (ACT, S2 = 1/(32767*GAMMA))
    pm2_k   = 256*(1 - th_k)          (ACT Copy scale/bias; = 512*p)
    w2_k    = pm2_k * ex_k            (DVE TT, into wc plane 3)
    wc planes 0..2 = w2*col, plane 3 = w2; one 4-plane add-tree
              -> cw = [csum'(3), wsum'] (f16, all x256)
    delta'  = exp(-zqmax*S2 + ln(512)) (ACT; f16, = 512*delta_ref)
    denom'  = max(wsum',1e-27) + delta'  (DVE STT, f32)
    rcp'    = exp(-ln(denom'))        (ACT Ln+Exp -> f16; Ln and Exp
              share the natural_log_exp table set: 2 loads/phase)
    t3'     = csum' + delta'          (DVE TT f16 2x, into out planes 0..2)
    rgb     = t3' * min(rcp', 60000)  (DVE STT in place; the min guards
              the ~1e-5 of pixels with denom' below f16 range)

All compute lives on DVE+ACT.  GPSIMD is intentionally unused: its SBUF
port is shared with the DVE, and measured contention slowed concurrent
dense fp16 DVE ops up to ~8x, wiping out any offload win.

Pipeline (phase u; sizes TS_PHASES, small first/last phases shorten
fill/drain): DVE iter u runs the z-stage for u+1 first (zq tree, zd),
then the q-path for u, then rgb for u-1 (the z-stage plus q-path cover
the ACT Ln/Exp latency of the reciprocal even in the short ramp/tail
iterations), then the main stage for u ending in denom and t3'.  ACT
runs th/qq2/pm2/ex/delta for u+1, then alpha/out-DMA/rcp.  SP streams
input DMAs (double-buffered, col one iteration behind zq/d so the next
phase's small tensors aren't queued behind a 3MB col transfer).
"""

import sys
from contextlib import ExitStack

import numpy as np

if "/opt/trn_rl_repo" not in sys.path:
    sys.path.insert(0, "/opt/trn_rl_repo")

SIGMA = 1e-4
GAMMA = 1e-4
ZNEAR = 1.0
ZFAR = 100.0
D = ZFAR - ZNEAR
ZQ = 32767.0                                   # z_inv quantization scale
S2 = 1.0 / (ZQ * GAMMA)                        # exp scale on zd
B_DELTA = float(np.log(512.0))                 # ln2 + ln(256) scaling

P = 128
K = 8
N_CORES = 8


def build_program(rows, TS):
    """TS: list of per-phase tile sizes (pixels per partition), sum == rows.
    Small first/last phases shorten pipeline fill/drain."""
    import concourse.bass as bass
    from concourse import mybir

    dt = mybir.dt
    f32 = dt.float32
    f16 = dt.float16
    i16 = dt.int16
    Alu = mybir.AluOpType
    Act = mybir.ActivationFunctionType

    assert sum(TS) == rows
    n = len(TS)
    off = [sum(TS[:t]) for t in range(n)]      # row offset of each phase
    T = max(TS)
    TK = T * K

    nc = bass.Bass()

    zq_d = nc.dram_tensor("zq", [P, rows * K], i16, kind="ExternalInput")
    ds_d = nc.dram_tensor("dists", [P, rows * K], f16, kind="ExternalInput")
    pc_d = nc.dram_tensor("pixel_colors", [P, rows * 3 * K], f16,
                          kind="ExternalInput")
    out_d = nc.dram_tensor("out", [P, rows * 4], f16, kind="ExternalOutput")

    # const AP for the Exp bias (Exp needs an AP bias; Copy takes imm)
    cb = nc.alloc_sbuf_tensor("c_bd", [P, 1], f32)
    nc.gpsimd.memset(cb.ap(), B_DELTA)
    nc.const_aps.aps[(f32, B_DELTA)] = cb.ap()
    nc.all_engine_barrier()

    with ExitStack() as ctx:
        def sb(name, w, dtype=f16):
            return ctx.enter_context(nc.sbuf_tensor(name, [P, w], dtype))

        zq = [sb(f"zq{j}", TK, i16) for j in range(2)]
        dth = [sb(f"dth{j}", TK) for j in range(2)]        # d, then th in place
        col = [sb(f"col{j}", TK * 3) for j in range(2)]
        qq2 = [sb(f"qq2{j}", TK) for j in range(2)]
        pm2 = sb("pm2", TK)                                # from ACT
        zdex = [sb(f"zdex{j}", TK) for j in range(2)]      # zd, then ex in place
        zmx4 = sb("zmx4", T * 4, i16)                      # lvl2 aliases [0:2T]
        zqmax = [sb(f"zqmax{j}", T) for j in range(2)]
        wcb = sb("wcb", TK * 4)                            # planes rgb + w2
        t4a = sb("t4a", T * 16)                            # lvl2 aliases [0:8T]
        qs4 = sb("qs4", T * 4)                             # lvl2 aliases [0:2T]
        prodq = [sb(f"prodq{j}", T) for j in range(2)]
        cw = [sb(f"cw{j}", T * 4) for j in range(2)]       # csum'*3, wsum'
        delta = [sb(f"delta{j}", T) for j in range(3)]
        denomn = sb("denomn", T, f32)
        rcpn = [sb(f"rcpn{j}", T) for j in range(2)]       # from ACT
        ot = [sb(f"ot{j}", T * 4) for j in range(2)]       # planes r,g,b,a
        warm = sb("warm", 1, f32)

        s_inz = ctx.enter_context(nc.semaphore("s_inz"))
        s_ind = ctx.enter_context(nc.semaphore("s_ind"))
        s_inc = ctx.enter_context(nc.semaphore("s_inc"))
        s_out = [ctx.enter_context(nc.semaphore(f"s_out{j}")) for j in range(2)]
        s_act = ctx.enter_context(nc.semaphore("s_act"))
        s_dve = ctx.enter_context(nc.semaphore("s_dve"))

        marks = {}

        def mk(eng, name, t, c):
            marks[(eng, name, t)] = c

        def out_done(t):
            return 16 * (t // 2 + 1)

        def v_kt(buf, Tl):
            return buf[:, 0:K * Tl].rearrange("p (k t) -> p k t", k=K)

        def v_ckt(buf, Tl):
            return buf[:, 0:3 * K * Tl].rearrange("p (c k t) -> p c k t",
                                                  c=3, k=K)

        # ---------------- SP: input DMAs, double-buffered -----------------
        # col[t] is issued one iteration late so zq/d of the next phase
        # (needed first by DVE/ACT) are not queued behind a 3MB col xfer
        def sched_sp(sp):
            for t in range(n + 1):
                if sp is None:
                    continue
                if t < n:
                    o, Tl = off[t], TS[t]
                    # phase 0: d first -- it unblocks the long serial ACT
                    # chain (th->qq2->pm2->ex), while zq only feeds the
                    # short z-stage
                    if t == 0:
                        sp.dma_start(out=dth[0][:, 0:K * Tl],
                                     in_=ds_d[:, o * K:(o + Tl) * K]
                                     ).then_inc(s_ind, 16)
                    if t >= 2:
                        sp.wait_ge(s_dve, marks[("d", "zd", t - 2)])
                    sp.dma_start(out=zq[t % 2][:, 0:K * Tl],
                                 in_=zq_d[:, o * K:(o + Tl) * K]
                                 ).then_inc(s_inz, 16)
                    if t != 0:
                        if t >= 2:
                            sp.wait_ge(s_act, marks[("a", "pm2", t - 2)])
                        sp.dma_start(out=dth[t % 2][:, 0:K * Tl],
                                     in_=ds_d[:, o * K:(o + Tl) * K]
                                     ).then_inc(s_ind, 16)
                tc = t - 1
                if 0 <= tc < n:
                    o, Tl = off[tc], TS[tc]
                    if tc >= 2:
                        sp.wait_ge(s_dve, marks[("d", "wc", tc - 2)])
                    sp.dma_start(out=col[tc % 2][:, 0:3 * K * Tl],
                                 in_=pc_d[:, o * 3 * K:(o + Tl) * 3 * K]
                                 ).then_inc(s_inc, 16)

        # ---------------- ACT ----------------
        def sched_act(act):
            c = 0
            if act is not None:
                act.activation(warm[:], warm[:], Act.Tanh, scale=1.0)
            for u in range(-1, n + 1):
                tz = u + 1
                if tz < n:
                    j = tz % 2
                    Tl = TS[tz]
                    # th in place over d
                    if act is not None:
                        act.wait_ge(s_ind, 16 * (tz + 1))
                        act.activation(dth[j][:, 0:K * Tl],
                                       dth[j][:, 0:K * Tl], Act.Tanh,
                                       scale=1.0 / (2.0 * SIGMA)
                                       ).then_inc(s_act, 1)
                    c += 1
                    mk("a", "th", tz, c)
                    if act is not None:
                        if tz >= 2:
                            act.wait_ge(s_dve, marks[("d", "q1", tz - 2)])
                        act.activation(qq2[j][:, 0:K * Tl],
                                       dth[j][:, 0:K * Tl], Act.Copy,
                                       scale=1.0, bias=1.0).then_inc(s_act, 1)
                    c += 1
                    mk("a", "qq2", tz, c)
                    # pm2 = 256*(1-th), single-buffered: wait for the
                    # previous phase's w2 to have consumed it
                    if act is not None:
                        if tz >= 1:
                            act.wait_ge(s_dve, marks[("d", "w2", tz - 1)])
                        act.activation(pm2[:, 0:K * Tl],
                                       dth[j][:, 0:K * Tl], Act.Copy,
                                       scale=-256.0, bias=256.0
                                       ).then_inc(s_act, 1)
                    c += 1
                    mk("a", "pm2", tz, c)
                    # ex in place over zd
                    if act is not None:
                        act.wait_ge(s_dve, marks[("d", "zd", tz)])
                        act.activation(zdex[j][:, 0:K * Tl],
                                       zdex[j][:, 0:K * Tl], Act.Exp,
                                       scale=S2).then_inc(s_act, 1)
                    c += 1
                    mk("a", "ex", tz, c)
                    if act is not None:
                        act.wait_ge(s_dve, marks[("d", "zqmax", tz)])
                        if tz >= 3:
                            act.wait_ge(s_dve, marks[("d", "t3", tz - 3)])
                        act.activation(delta[tz % 3][:, 0:Tl],
                                       zqmax[j][:, 0:Tl], Act.Exp,
                                       scale=-S2, bias=B_DELTA
                                       ).then_inc(s_act, 1)
                    c += 1
                    mk("a", "delta", tz, c)
                ta = u
                if 0 <= ta < n:
                    Tl = TS[ta]
                    if act is not None:
                        act.wait_ge(s_dve, marks[("d", "prodq", ta)])
                        if ta >= 2:
                            act.wait_ge(s_out[ta % 2], out_done(ta - 2))
                        ot_v = ot[ta % 2][:, 0:4 * Tl].rearrange(
                            "p (c t) -> p c t", c=4)
                        act.activation(ot_v[:, 3:4, :],
                                       prodq[ta % 2][:, 0:Tl].unsqueeze(1),
                                       Act.Copy, scale=-1.0 / 256.0, bias=1.0
                                       ).then_inc(s_act, 1)
                    c += 1
                    mk("a", "alpha", ta, c)
                to = u - 1
                if 0 <= to < n:
                    if act is not None:
                        oo, Tl = off[to], TS[to]
                        act.wait_ge(s_dve, marks[("d", "rgb", to)])
                        act.dma_start(
                            out=out_d[:, oo * 4:(oo + Tl) * 4],
                            in_=ot[to % 2][:, 0:4 * Tl]
                        ).then_inc(s_out[to % 2], 16)
                ta = u
                if 0 <= ta < n:
                    Tl = TS[ta]
                    if act is not None:
                        act.wait_ge(s_dve, marks[("d", "denom", ta)])
                        # rcp' = exp(-ln(denom')) in f16; Ln+Exp share the
                        # natural_log_exp_and_others table set
                        act.activation(denomn[:, 0:Tl], denomn[:, 0:Tl],
                                       Act.Ln, scale=1.0)
                        act.activation(rcpn[ta % 2][:, 0:Tl],
                                       denomn[:, 0:Tl], Act.Exp,
                                       scale=-1.0).then_inc(s_act, 1)
                    c += 1
                    mk("a", "rcp", ta, c)
            if act is not None:
                act.wait_ge(s_out[0], 16 * ((n + 1) // 2))
                act.wait_ge(s_out[1], 16 * (n // 2))

        # ---------------- DVE ----------------
        def sched_dve(dve):
            c = 0
            def q_path(t):
                nonlocal c
                if 0 <= t < n:
                    j = t % 2
                    Tl = TS[t]
                    if dve is not None:
                        dve.wait_ge(s_act, marks[("a", "qq2", t)])
                        q_v = v_kt(qq2[j], Tl)
                        q4 = qs4[:, 0:4 * Tl].rearrange(
                            "p (k t) -> p k t", k=4)
                        dve.tensor_tensor(out=q4, in0=q_v[:, 0:4, :],
                                          in1=q_v[:, 4:8, :],
                                          op=Alu.mult).then_inc(s_dve, 1)
                    c += 1
                    mk("d", "q1", t, c)
                    if dve is not None:
                        dve.tensor_tensor(out=q4[:, 0:2, :],
                                          in0=q4[:, 0:2, :],
                                          in1=q4[:, 2:4, :], op=Alu.mult)
                        if t >= 2:
                            dve.wait_ge(s_act, marks[("a", "alpha", t - 2)])
                        dve.tensor_tensor(out=prodq[j][:, 0:Tl].unsqueeze(1),
                                          in0=q4[:, 0:1, :],
                                          in1=q4[:, 1:2, :],
                                          op=Alu.mult).then_inc(s_dve, 1)
                    c += 1
                    mk("d", "prodq", t, c)

            for u in range(-1, n + 1):
                if u == 0:
                    # fill: q[0]'s input is ready before zq[1] lands
                    q_path(0)
                tz = u + 1
                if 0 <= tz < n:
                    j = tz % 2
                    Tl = TS[tz]
                    emit = dve is not None
                    if emit:
                        dve.wait_ge(s_inz, 16 * (tz + 1))
                        zq_v = v_kt(zq[j], Tl)
                        zx4 = zmx4[:, 0:4 * Tl].rearrange(
                            "p (k t) -> p k t", k=4)
                        dve.tensor_tensor(out=zx4, in0=zq_v[:, 0:4, :],
                                          in1=zq_v[:, 4:8, :], op=Alu.max)
                        dve.tensor_tensor(out=zx4[:, 0:2, :],
                                          in0=zx4[:, 0:2, :],
                                          in1=zx4[:, 2:4, :], op=Alu.max)
                        if tz >= 2:
                            dve.wait_ge(s_act, marks[("a", "delta", tz - 2)])
                        dve.tensor_tensor(out=zqmax[j][:, 0:Tl].unsqueeze(1),
                                          in0=zx4[:, 0:1, :],
                                          in1=zx4[:, 1:2, :],
                                          op=Alu.max).then_inc(s_dve, 1)
                    c += 1
                    mk("d", "zqmax", tz, c)
                    if emit:
                        dve.tensor_tensor(
                            out=v_kt(zdex[j], Tl),
                            in0=v_kt(zq[j], Tl),
                            in1=zqmax[j][:, 0:Tl].unsqueeze(1)
                                .broadcast_to((P, K, Tl)),
                            op=Alu.subtract).then_inc(s_dve, 1)
                    c += 1
                    mk("d", "zd", tz, c)
                # q-path for u: no reciprocal dependency, so it covers
                # the ACT Ln/Exp latency before rgb[u-1] below (u==0 ran
                # it before the z-stage above, during the zq[1] DMA wait)
                if u != 0:
                    q_path(u)
                tr = u - 1
                if 0 <= tr < n:
                    jr = tr % 2
                    Tl = TS[tr]
                    if dve is not None:
                        dve.wait_ge(s_act, marks[("a", "rcp", tr)])
                        otr_v = ot[jr][:, 0:4 * Tl].rearrange(
                            "p (c t) -> p c t", c=4)
                        dve.scalar_tensor_tensor(
                            out=otr_v[:, 0:3, :],
                            in0=rcpn[jr][:, 0:Tl].unsqueeze(1)
                                .broadcast_to((P, 3, Tl)),
                            scalar=60000.0, in1=otr_v[:, 0:3, :],
                            op0=Alu.min, op1=Alu.mult).then_inc(s_dve, 1)
                    c += 1
                    mk("d", "rgb", tr, c)
                t = u
                if not (0 <= t < n):
                    continue
                j = t % 2
                Tl = TS[t]
                emit = dve is not None
                if emit:
                    dve.wait_ge(s_act, marks[("a", "ex", t)])
                    dve.wait_ge(s_act, marks[("a", "pm2", t)])
                    wcv = wcb[:, 0:4 * K * Tl].rearrange(
                        "p (c k t) -> p c k t", c=4, k=K)
                    dve.tensor_tensor(out=wcv[:, 3, :, :],
                                      in0=pm2[:, 0:K * Tl].rearrange(
                                          "p (k t) -> p k t", k=K),
                                      in1=v_kt(zdex[j], Tl),
                                      op=Alu.mult).then_inc(s_dve, 1)
                c += 1
                mk("d", "w2", t, c)
                if emit:
                    dve.wait_ge(s_inc, 16 * (t + 1))
                    dve.tensor_tensor(
                        out=wcv[:, 0:3, :, :],
                        in0=wcv[:, 3:4, :, :].broadcast_to((P, 3, K, Tl)),
                        in1=v_ckt(col[j], Tl),
                        op=Alu.mult).then_inc(s_dve, 1)
                c += 1
                mk("d", "wc", t, c)
                if emit:
                    t4 = t4a[:, 0:16 * Tl].rearrange(
                        "p (c k t) -> p c k t", c=4, k=4)
                    dve.tensor_tensor(out=t4, in0=wcv[:, :, 0:4, :],
                                      in1=wcv[:, :, 4:8, :], op=Alu.add)
                    dve.tensor_tensor(out=t4[:, :, 0:2, :],
                                      in0=t4[:, :, 0:2, :],
                                      in1=t4[:, :, 2:4, :], op=Alu.add)
                    cw_v = cw[j][:, 0:4 * Tl].rearrange(
                        "p (c t) -> p c t", c=4)
                    dve.tensor_tensor(out=cw_v,
                                      in0=t4[:, :, 0, :],
                                      in1=t4[:, :, 1, :],
                                      op=Alu.add).then_inc(s_dve, 1)
                c += 1
                mk("d", "cw", t, c)
                if emit:
                    dve.wait_ge(s_act, marks[("a", "delta", t)])
                    if t >= 1:
                        dve.wait_ge(s_act, marks[("a", "rcp", t - 1)])
                    dve.scalar_tensor_tensor(
                        out=denomn[:, 0:Tl], in0=cw_v[:, 3, :], scalar=1e-27,
                        in1=delta[t % 3][:, 0:Tl], op0=Alu.max, op1=Alu.add,
                    ).then_inc(s_dve, 1)
                c += 1
                mk("d", "denom", t, c)
                if emit:
                    if t >= 2:
                        dve.wait_ge(s_out[j], out_done(t - 2))
                    ot_v = ot[j][:, 0:4 * Tl].rearrange(
                        "p (c t) -> p c t", c=4)
                    dve.tensor_tensor(
                        out=ot_v[:, 0:3, :], in0=cw_v[:, 0:3, :],
                        in1=delta[t % 3][:, 0:Tl].unsqueeze(1)
                            .broadcast_to((P, 3, Tl)),
                        op=Alu.add).then_inc(s_dve, 1)
                c += 1
                mk("d", "t3", t, c)

        sched_sp(None)
        sched_act(None)
        sched_dve(None)

        blk = ctx.enter_context(nc.Block())

        @blk.sync
        def _(sp):
            sched_sp(sp)

        @blk.scalar
        def _(act):
            sched_act(act)

        @blk.vector
        def _(dve):
            sched_dve(dve)

    return nc


_CACHE = {}

# small first/last phases shorten pipeline fill/drain
TS_PHASES = (128, 512, 512, 512, 384)


def _get_program(rows=2048, TS=TS_PHASES):
    key = (rows, TS)
    if key not in _CACHE:
        _CACHE[key] = build_program(rows, list(TS))
    return _CACHE[key]


def _kmaj(a, TS, inner):
    """[P, rows, K, inner...] -> per-phase k-major planar, flattened."""
    parts = []
    o = 0
    for Tl in TS:
        s = a[:, o:o + Tl]                     # [P, Tl, K] or [P, Tl, K, 3]
        if s.ndim == 3:
            s = s.transpose(0, 2, 1)           # [P, K, Tl]
        else:
            s = s.transpose(0, 3, 2, 1)        # [P, 3, K, Tl]
        parts.append(np.ascontiguousarray(s).reshape(P, -1))
        o += Tl
    return np.concatenate(parts, axis=1)


def _prep_core(zb, ds, pf, pc, TS):
    """Host-side repack for one core: returns dict of DRAM arrays."""
    mask = pf >= 0                                        # [P, rows, K]
    z_inv = (ZFAR - zb) * (1.0 / D)
    np.clip(z_inv, 0.0, 1.0, out=z_inv)
    zq = np.rint(z_inv * ZQ).astype(np.int16)
    zq[~mask] = 0
    d16 = ds.astype(np.float16)
    d16[~mask] = np.float16(30000.0)
    return {
        "zq": _kmaj(zq, TS, 1),
        "dists": _kmaj(d16, TS, 1),
        "pixel_colors": _kmaj(pc.astype(np.float16), TS, 3),
    }


def _run(pixel_colors, zbuf, dists, pix_to_face, trace=False):
    from concourse.bass_utils import run_bass_kernel_spmd

    N, H, W, Kk = zbuf.shape
    assert (N, H, W, Kk) == (8, 512, 512, 8), (N, H, W, Kk)
    rows = H * W // P  # 2048
    TS = TS_PHASES
    assert sum(TS) == rows

    nc = _get_program(rows=rows, TS=TS)

    zb = np.asarray(zbuf, dtype=np.float32)
    ds = np.asarray(dists, dtype=np.float32)
    pf = np.asarray(pix_to_face)
    pc = np.asarray(pixel_colors, dtype=np.float32)

    in_maps = []
    for i in range(N_CORES):
        in_maps.append(_prep_core(
            zb[i].reshape(P, rows, K),
            ds[i].reshape(P, rows, K),
            pf[i].reshape(P, rows, K),
            pc[i].reshape(P, rows, K, 3),
            TS,
        ))

    res = run_bass_kernel_spmd(
        nc, in_maps, core_ids=list(range(N_CORES)), trace=trace
    )
    outs = []
    for i in range(N_CORES):
        o = res.results[i]["out"].astype(np.float32)
        # per-phase planar [P, 4, Tl] -> [P, rows, 4]
        parts = []
        oo = 0
        for Tl in TS:
            chunk = o[:, oo * 4:(oo + Tl) * 4].reshape(P, 4, Tl)
            parts.append(chunk.transpose(0, 2, 1))
            oo += Tl
        full = np.concatenate(parts, axis=1).reshape(H, W, 4)
        outs.append(full)
    return np.stack(outs, axis=0), res


def kernel(pixel_colors, zbuf, dists, pix_to_face):
    out, _ = _run(pixel_colors, zbuf, dists, pix_to_face, trace=False)
    return out


# revision 64
# speedup vs baseline: 1.0270x; 1.0270x over previous
"""Trainium2 Bass kernel for softmax RGB blend (pytorch3d NoLightShader).

Full inputs (N=8, H=512, W=512, K=8) are sharded batch-wise across 8
NeuronCores (one image per core); the blend is per-pixel so no cross-core
communication is needed.

Host-side input repack (per core, pure layout/dtype transforms):
  - mask folded into dists:  d' = masked ? +30000 : d   (fp16)
      (tanh(d'*5000) = 1  ->  q = 1, p = 0, exactly the masked case)
  - mask folded into z via quantization:
      zq = round((ZFAR - z)/D * 32767) * mask   (int16; 0 when masked,
      matching the reference's masked z_inv == 0 exactly)
  - colors fp16, k-major planar per phase:  [P, n, 3, K, T]
  - zq/d' k-major per phase: [P, n, K, T]
Output is written planar [P, n, 4, T] (r,g,b,a planes) and untransposed
on the host.

Math per pixel (K faces); everything is scaled by 256 (folded into pm2
and the delta bias) so that rcp = 1/denom' stays in fp16 range:
    th_k    = tanh(d_k*5000)          (ACT; masked -> 1)
    qq2_k   = 1 + th_k    (= 2q)      (ACT Copy;  prodq = prod_k qq2_k)
    alpha   = 1 - prodq/256           (ACT Copy, into out plane 3)
    zqmax   = max_k zq_k  (int16 TT-max tree -> f16; f16 rounding of
              zqmax is common-mode across k and cancels in the blend)
    zd_k    = zq_k - zqmax            (int16 - f16 bcast -> f16, 2x)
    ex_k    = exp(zd_k * S2)          (ACT, S2 = 1/(32767*GAMMA))
    pm2_k   = 256*(1 - th_k)          (ACT Copy scale/bias; = 512*p)
    w2_k    = pm2_k * ex_k            (DVE TT, into wc plane 3)
    wc planes 0..2 = w2*col, plane 3 = w2; one 4-plane add-tree
              -> cw = [csum'(3), wsum'] (f16, all x256)
    delta'  = exp(-zqmax*S2 + ln(512)) (ACT; f16, = 512*delta_ref)
    denom'  = max(wsum',1e-27) + delta'  (DVE STT, f32)
    rcp'    = exp(-ln(denom'))        (ACT Ln+Exp -> f16; Ln and Exp
              share the natural_log_exp table set: 2 loads/phase)
    t3'     = csum' + delta'          (DVE TT f16 2x, into out planes 0..2)
    rgb     = t3' * min(rcp', 60000)  (DVE STT in place; the min guards
              the ~1e-5 of pixels with denom' below f16 range)

All compute lives on DVE+ACT.  GPSIMD is intentionally unused: its SBUF
port is shared with the DVE, and measured contention slowed concurrent
dense fp16 DVE ops up to ~8x, wiping out any offload win.

Pipeline (phase u; sizes TS_PHASES, small first/last phases shorten
fill/drain): DVE iter u runs the z-stage for u+1 first (zq tree, zd),
then the q-path for u, then rgb for u-1 (the z-stage plus q-path cover
the ACT Ln/Exp latency of the reciprocal even in the short ramp/tail
iterations), then the main stage for u ending in denom and t3'.  ACT
runs th/qq2/pm2/ex/delta for u+1, then alpha/out-DMA/rcp.  SP streams
input DMAs (double-buffered, col one iteration behind zq/d so the next
phase's small tensors aren't queued behind a 3MB col transfer).
"""

import sys
from contextlib import ExitStack

import numpy as np

if "/opt/trn_rl_repo" not in sys.path:
    sys.path.insert(0, "/opt/trn_rl_repo")

SIGMA = 1e-4
GAMMA = 1e-4
ZNEAR = 1.0
ZFAR = 100.0
D = ZFAR - ZNEAR
ZQ = 32767.0                                   # z_inv quantization scale
S2 = 1.0 / (ZQ * GAMMA)                        # exp scale on zd
B_DELTA = float(np.log(512.0))                 # ln2 + ln(256) scaling

P = 128
K = 8
N_CORES = 8


def build_program(rows, TS):
    """TS: list of per-phase tile sizes (pixels per partition), sum == rows.
    Small first/last phases shorten pipeline fill/drain."""
    import concourse.bass as bass
    from concourse import mybir

    dt = mybir.dt
    f32 = dt.float32
    f16 = dt.float16
    i16 = dt.int16
    Alu = mybir.AluOpType
    Act = mybir.ActivationFunctionType

    assert sum(TS) == rows
    n = len(TS)
    off = [sum(TS[:t]) for t in range(n)]      # row offset of each phase
    T = max(TS)
    TK = T * K

    nc = bass.Bass()

    zq_d = nc.dram_tensor("zq", [P, rows * K], i16, kind="ExternalInput")
    ds_d = nc.dram_tensor("dists", [P, rows * K], f16, kind="ExternalInput")
    pc_d = nc.dram_tensor("pixel_colors", [P, rows * 3 * K], f16,
                          kind="ExternalInput")
    out_d = nc.dram_tensor("out", [P, rows * 4], f16, kind="ExternalOutput")

    # const AP for the Exp bias (Exp needs an AP bias; Copy takes imm)
    cb = nc.alloc_sbuf_tensor("c_bd", [P, 1], f32)
    nc.gpsimd.memset(cb.ap(), B_DELTA)
    nc.const_aps.aps[(f32, B_DELTA)] = cb.ap()
    nc.all_engine_barrier()

    with ExitStack() as ctx:
        def sb(name, w, dtype=f16):
            return ctx.enter_context(nc.sbuf_tensor(name, [P, w], dtype))

        zq = [sb(f"zq{j}", TK, i16) for j in range(2)]
        dth = [sb(f"dth{j}", TK) for j in range(2)]        # d, then th in place
        col = [sb(f"col{j}", TK * 3) for j in range(2)]
        qq2 = [sb(f"qq2{j}", TK) for j in range(2)]
        pm2 = sb("pm2", TK)                                # from ACT
        zdex = [sb(f"zdex{j}", TK) for j in range(2)]      # zd, then ex in place
        zmx4 = sb("zmx4", T * 4, i16)                      # lvl2 aliases [0:2T]
        zqmax = [sb(f"zqmax{j}", T) for j in range(2)]
        wcb = sb("wcb", TK * 4)                            # planes rgb + w2
        t4a = sb("t4a", T * 16)                            # lvl2 aliases [0:8T]
        qs4 = sb("qs4", T * 4)                             # lvl2 aliases [0:2T]
        prodq = [sb(f"prodq{j}", T) for j in range(2)]
        cw = [sb(f"cw{j}", T * 4) for j in range(2)]       # csum'*3, wsum'
        delta = [sb(f"delta{j}", T) for j in range(3)]
        denomn = sb("denomn", T, f32)
        rcpn = [sb(f"rcpn{j}", T) for j in range(2)]       # from ACT
        ot = [sb(f"ot{j}", T * 4) for j in range(2)]       # planes r,g,b,a
        warm = sb("warm", 1, f32)

        s_inz = ctx.enter_context(nc.semaphore("s_inz"))
        s_ind = ctx.enter_context(nc.semaphore("s_ind"))
        s_inc = ctx.enter_context(nc.semaphore("s_inc"))
        s_out = [ctx.enter_context(nc.semaphore(f"s_out{j}")) for j in range(2)]
        s_act = ctx.enter_context(nc.semaphore("s_act"))
        s_dve = ctx.enter_context(nc.semaphore("s_dve"))

        marks = {}

        def mk(eng, name, t, c):
            marks[(eng, name, t)] = c

        def out_done(t):
            return 16 * (t // 2 + 1)

        def v_kt(buf, Tl):
            return buf[:, 0:K * Tl].rearrange("p (k t) -> p k t", k=K)

        def v_ckt(buf, Tl):
            return buf[:, 0:3 * K * Tl].rearrange("p (c k t) -> p c k t",
                                                  c=3, k=K)

        # ---------------- SP: input DMAs, double-buffered -----------------
        # col[t] is issued one iteration late so zq/d of the next phase
        # (needed first by DVE/ACT) are not queued behind a 3MB col xfer
        def sched_sp(sp):
            for t in range(n + 1):
                if sp is None:
                    continue
                if t < n:
                    o, Tl = off[t], TS[t]
                    # phase 0: d first -- it unblocks the long serial ACT
                    # chain (th->qq2->pm2->ex), while zq only feeds the
                    # short z-stage
                    if t == 0:
                        sp.dma_start(out=dth[0][:, 0:K * Tl],
                                     in_=ds_d[:, o * K:(o + Tl) * K]
                                     ).then_inc(s_ind, 16)
                    if t >= 2:
                        sp.wait_ge(s_dve, marks[("d", "zd", t - 2)])
                    sp.dma_start(out=zq[t % 2][:, 0:K * Tl],
                                 in_=zq_d[:, o * K:(o + Tl) * K]
                                 ).then_inc(s_inz, 16)
                    if t != 0:
                        if t >= 2:
                            sp.wait_ge(s_act, marks[("a", "pm2", t - 2)])
                        sp.dma_start(out=dth[t % 2][:, 0:K * Tl],
                                     in_=ds_d[:, o * K:(o + Tl) * K]
                                     ).then_inc(s_ind, 16)
                tc = t - 1
                if 0 <= tc < n:
                    o, Tl = off[tc], TS[tc]
                    if tc >= 2:
                        sp.wait_ge(s_dve, marks[("d", "wc", tc - 2)])
                    sp.dma_start(out=col[tc % 2][:, 0:3 * K * Tl],
                                 in_=pc_d[:, o * 3 * K:(o + Tl) * 3 * K]
                                 ).then_inc(s_inc, 16)

        # ---------------- ACT ----------------
        def sched_act(act):
            c = 0
            if act is not None:
                act.activation(warm[:], warm[:], Act.Tanh, scale=1.0)
            for u in range(-1, n + 1):
                tz = u + 1
                if tz < n:
                    j = tz % 2
                    Tl = TS[tz]
                    # th in place over d
                    if act is not None:
                        act.wait_ge(s_ind, 16 * (tz + 1))
                        act.activation(dth[j][:, 0:K * Tl],
                                       dth[j][:, 0:K * Tl], Act.Tanh,
                                       scale=1.0 / (2.0 * SIGMA)
                                       ).then_inc(s_act, 1)
                    c += 1
                    mk("a", "th", tz, c)
                    if act is not None:
                        if tz >= 2:
                            act.wait_ge(s_dve, marks[("d", "q1", tz - 2)])
                        act.activation(qq2[j][:, 0:K * Tl],
                                       dth[j][:, 0:K * Tl], Act.Copy,
                                       scale=1.0, bias=1.0).then_inc(s_act, 1)
                    c += 1
                    mk("a", "qq2", tz, c)
                    # pm2 = 256*(1-th), single-buffered: wait for the
                    # previous phase's w2 to have consumed it
                    if act is not None:
                        if tz >= 1:
                            act.wait_ge(s_dve, marks[("d", "w2", tz - 1)])
                        act.activation(pm2[:, 0:K * Tl],
                                       dth[j][:, 0:K * Tl], Act.Copy,
                                       scale=-256.0, bias=256.0
                                       ).then_inc(s_act, 1)
                    c += 1
                    mk("a", "pm2", tz, c)
                    # ex in place over zd
                    if act is not None:
                        act.wait_ge(s_dve, marks[("d", "zd", tz)])
                        act.activation(zdex[j][:, 0:K * Tl],
                                       zdex[j][:, 0:K * Tl], Act.Exp,
                                       scale=S2).then_inc(s_act, 1)
                    c += 1
                    mk("a", "ex", tz, c)
                    if act is not None:
                        act.wait_ge(s_dve, marks[("d", "zqmax", tz)])
                        if tz >= 3:
                            act.wait_ge(s_dve, marks[("d", "t3", tz - 3)])
                        act.activation(delta[tz % 3][:, 0:Tl],
                                       zqmax[j][:, 0:Tl], Act.Exp,
                                       scale=-S2, bias=B_DELTA
                                       ).then_inc(s_act, 1)
                    c += 1
                    mk("a", "delta", tz, c)
                ta = u
                if 0 <= ta < n:
                    Tl = TS[ta]
                    if act is not None:
                        act.wait_ge(s_dve, marks[("d", "prodq", ta)])
                        if ta >= 2:
                            act.wait_ge(s_out[ta % 2], out_done(ta - 2))
                        ot_v = ot[ta % 2][:, 0:4 * Tl].rearrange(
                            "p (c t) -> p c t", c=4)
                        act.activation(ot_v[:, 3:4, :],
                                       prodq[ta % 2][:, 0:Tl].unsqueeze(1),
                                       Act.Copy, scale=-1.0 / 256.0, bias=1.0
                                       ).then_inc(s_act, 1)
                    c += 1
                    mk("a", "alpha", ta, c)
                to = u - 1
                if 0 <= to < n:
                    if act is not None:
                        oo, Tl = off[to], TS[to]
                        act.wait_ge(s_dve, marks[("d", "rgb", to)])
                        act.dma_start(
                            out=out_d[:, oo * 4:(oo + Tl) * 4],
                            in_=ot[to % 2][:, 0:4 * Tl]
                        ).then_inc(s_out[to % 2], 16)
                ta = u
                if 0 <= ta < n:
                    Tl = TS[ta]
                    if act is not None:
                        act.wait_ge(s_dve, marks[("d", "denom", ta)])
                        # rcp' = exp(-ln(denom')) in f16; Ln+Exp share the
                        # natural_log_exp_and_others table set
                        act.activation(denomn[:, 0:Tl], denomn[:, 0:Tl],
                                       Act.Ln, scale=1.0)
                        act.activation(rcpn[ta % 2][:, 0:Tl],
                                       denomn[:, 0:Tl], Act.Exp,
                                       scale=-1.0).then_inc(s_act, 1)
                    c += 1
                    mk("a", "rcp", ta, c)
            if act is not None:
                act.wait_ge(s_out[0], 16 * ((n + 1) // 2))
                act.wait_ge(s_out[1], 16 * (n // 2))

        # ---------------- DVE ----------------
        def sched_dve(dve):
            c = 0
            for u in range(-1, n + 1):
                tz = u + 1
                if 0 <= tz < n:
                    j = tz % 2
                    Tl = TS[tz]
                    emit = dve is not None
                    if emit:
                        dve.wait_ge(s_inz, 16 * (tz + 1))
                        zq_v = v_kt(zq[j], Tl)
                        zx4 = zmx4[:, 0:4 * Tl].rearrange(
                            "p (k t) -> p k t", k=4)
                        dve.tensor_tensor(out=zx4, in0=zq_v[:, 0:4, :],
                                          in1=zq_v[:, 4:8, :], op=Alu.max)
                        dve.tensor_tensor(out=zx4[:, 0:2, :],
                                          in0=zx4[:, 0:2, :],
                                          in1=zx4[:, 2:4, :], op=Alu.max)
                        if tz >= 2:
                            dve.wait_ge(s_act, marks[("a", "delta", tz - 2)])
                        dve.tensor_tensor(out=zqmax[j][:, 0:Tl].unsqueeze(1),
                                          in0=zx4[:, 0:1, :],
                                          in1=zx4[:, 1:2, :],
                                          op=Alu.max).then_inc(s_dve, 1)
                    c += 1
                    mk("d", "zqmax", tz, c)
                    if emit:
                        dve.tensor_tensor(
                            out=v_kt(zdex[j], Tl),
                            in0=v_kt(zq[j], Tl),
                            in1=zqmax[j][:, 0:Tl].unsqueeze(1)
                                .broadcast_to((P, K, Tl)),
                            op=Alu.subtract).then_inc(s_dve, 1)
                    c += 1
                    mk("d", "zd", tz, c)
                # q-path for u first: it does not depend on the
                # reciprocal, so it covers the ACT Ln/Exp latency before
                # rgb[u-1] below (the z-stage alone is too short in the
                # ramp/tail iterations)
                t = u
                if 0 <= t < n:
                    j = t % 2
                    Tl = TS[t]
                    if dve is not None:
                        dve.wait_ge(s_act, marks[("a", "qq2", t)])
                        q_v = v_kt(qq2[j], Tl)
                        q4 = qs4[:, 0:4 * Tl].rearrange(
                            "p (k t) -> p k t", k=4)
                        dve.tensor_tensor(out=q4, in0=q_v[:, 0:4, :],
                                          in1=q_v[:, 4:8, :],
                                          op=Alu.mult).then_inc(s_dve, 1)
                    c += 1
                    mk("d", "q1", t, c)
                    if dve is not None:
                        dve.tensor_tensor(out=q4[:, 0:2, :],
                                          in0=q4[:, 0:2, :],
                                          in1=q4[:, 2:4, :], op=Alu.mult)
                        if t >= 2:
                            dve.wait_ge(s_act, marks[("a", "alpha", t - 2)])
                        dve.tensor_tensor(out=prodq[j][:, 0:Tl].unsqueeze(1),
                                          in0=q4[:, 0:1, :],
                                          in1=q4[:, 1:2, :],
                                          op=Alu.mult).then_inc(s_dve, 1)
                    c += 1
                    mk("d", "prodq", t, c)
                tr = u - 1
                if 0 <= tr < n:
                    jr = tr % 2
                    Tl = TS[tr]
                    if dve is not None:
                        dve.wait_ge(s_act, marks[("a", "rcp", tr)])
                        otr_v = ot[jr][:, 0:4 * Tl].rearrange(
                            "p (c t) -> p c t", c=4)
                        dve.scalar_tensor_tensor(
                            out=otr_v[:, 0:3, :],
                            in0=rcpn[jr][:, 0:Tl].unsqueeze(1)
                                .broadcast_to((P, 3, Tl)),
                            scalar=60000.0, in1=otr_v[:, 0:3, :],
                            op0=Alu.min, op1=Alu.mult).then_inc(s_dve, 1)
                    c += 1
                    mk("d", "rgb", tr, c)
                t = u
                if not (0 <= t < n):
                    continue
                j = t % 2
                Tl = TS[t]
                emit = dve is not None
                if emit:
                    dve.wait_ge(s_act, marks[("a", "ex", t)])
                    dve.wait_ge(s_act, marks[("a", "pm2", t)])
                    wcv = wcb[:, 0:4 * K * Tl].rearrange(
                        "p (c k t) -> p c k t", c=4, k=K)
                    dve.tensor_tensor(out=wcv[:, 3, :, :],
                                      in0=pm2[:, 0:K * Tl].rearrange(
                                          "p (k t) -> p k t", k=K),
                                      in1=v_kt(zdex[j], Tl),
                                      op=Alu.mult).then_inc(s_dve, 1)
                c += 1
                mk("d", "w2", t, c)
                if emit:
                    dve.wait_ge(s_inc, 16 * (t + 1))
                    dve.tensor_tensor(
                        out=wcv[:, 0:3, :, :],
                        in0=wcv[:, 3:4, :, :].broadcast_to((P, 3, K, Tl)),
                        in1=v_ckt(col[j], Tl),
                        op=Alu.mult).then_inc(s_dve, 1)
                c += 1
                mk("d", "wc", t, c)
                if emit:
                    t4 = t4a[:, 0:16 * Tl].rearrange(
                        "p (c k t) -> p c k t", c=4, k=4)
                    dve.tensor_tensor(out=t4, in0=wcv[:, :, 0:4, :],
                                      in1=wcv[:, :, 4:8, :], op=Alu.add)
                    dve.tensor_tensor(out=t4[:, :, 0:2, :],
                                      in0=t4[:, :, 0:2, :],
                                      in1=t4[:, :, 2:4, :], op=Alu.add)
                    cw_v = cw[j][:, 0:4 * Tl].rearrange(
                        "p (c t) -> p c t", c=4)
                    dve.tensor_tensor(out=cw_v,
                                      in0=t4[:, :, 0, :],
                                      in1=t4[:, :, 1, :],
                                      op=Alu.add).then_inc(s_dve, 1)
                c += 1
                mk("d", "cw", t, c)
                if emit:
                    dve.wait_ge(s_act, marks[("a", "delta", t)])
                    if t >= 1:
                        dve.wait_ge(s_act, marks[("a", "rcp", t - 1)])
                    dve.scalar_tensor_tensor(
                        out=denomn[:, 0:Tl], in0=cw_v[:, 3, :], scalar=1e-27,
                        in1=delta[t % 3][:, 0:Tl], op0=Alu.max, op1=Alu.add,
                    ).then_inc(s_dve, 1)
                c += 1
                mk("d", "denom", t, c)
                if emit:
                    if t >= 2:
                        dve.wait_ge(s_out[j], out_done(t - 2))
                    ot_v = ot[j][:, 0:4 * Tl].rearrange(
                        "p (c t) -> p c t", c=4)
                    dve.tensor_tensor(
                        out=ot_v[:, 0:3, :], in0=cw_v[:, 0:3, :],
                        in1=delta[t % 3][:, 0:Tl].unsqueeze(1)
                            .broadcast_to((P, 3, Tl)),
                        op=Alu.add).then_inc(s_dve, 1)
                c += 1
                mk("d", "t3", t, c)

        sched_sp(None)
        sched_act(None)
        sched_dve(None)

        blk = ctx.enter_context(nc.Block())

        @blk.sync
        def _(sp):
            sched_sp(sp)

        @blk.scalar
        def _(act):
            sched_act(act)

        @blk.vector
        def _(dve):
            sched_dve(dve)

    return nc


_CACHE = {}

# small first/last phases shorten pipeline fill/drain
TS_PHASES = (256, 512, 512, 512, 256)


def _get_program(rows=2048, TS=TS_PHASES):
    key = (rows, TS)
    if key not in _CACHE:
        _CACHE[key] = build_program(rows, list(TS))
    return _CACHE[key]


def _kmaj(a, TS, inner):
    """[P, rows, K, inner...] -> per-phase k-major planar, flattened."""
    parts = []
    o = 0
    for Tl in TS:
        s = a[:, o:o + Tl]                     # [P, Tl, K] or [P, Tl, K, 3]
        if s.ndim == 3:
            s = s.transpose(0, 2, 1)           # [P, K, Tl]
        else:
            s = s.transpose(0, 3, 2, 1)        # [P, 3, K, Tl]
        parts.append(np.ascontiguousarray(s).reshape(P, -1))
        o += Tl
    return np.concatenate(parts, axis=1)


def _prep_core(zb, ds, pf, pc, TS):
    """Host-side repack for one core: returns dict of DRAM arrays."""
    mask = pf >= 0                                        # [P, rows, K]
    z_inv = (ZFAR - zb) * (1.0 / D)
    np.clip(z_inv, 0.0, 1.0, out=z_inv)
    zq = np.rint(z_inv * ZQ).astype(np.int16)
    zq[~mask] = 0
    d16 = ds.astype(np.float16)
    d16[~mask] = np.float16(30000.0)
    return {
        "zq": _kmaj(zq, TS, 1),
        "dists": _kmaj(d16, TS, 1),
        "pixel_colors": _kmaj(pc.astype(np.float16), TS, 3),
    }


def _run(pixel_colors, zbuf, dists, pix_to_face, trace=False):
    from concourse.bass_utils import run_bass_kernel_spmd

    N, H, W, Kk = zbuf.shape
    assert (N, H, W, Kk) == (8, 512, 512, 8), (N, H, W, Kk)
    rows = H * W // P  # 2048
    TS = TS_PHASES
    assert sum(TS) == rows

    nc = _get_program(rows=rows, TS=TS)

    zb = np.asarray(zbuf, dtype=np.float32)
    ds = np.asarray(dists, dtype=np.float32)
    pf = np.asarray(pix_to_face)
    pc = np.asarray(pixel_colors, dtype=np.float32)

    in_maps = []
    for i in range(N_CORES):
        in_maps.append(_prep_core(
            zb[i].reshape(P, rows, K),
            ds[i].reshape(P, rows, K),
            pf[i].reshape(P, rows, K),
            pc[i].reshape(P, rows, K, 3),
            TS,
        ))

    res = run_bass_kernel_spmd(
        nc, in_maps, core_ids=list(range(N_CORES)), trace=trace
    )
    outs = []
    for i in range(N_CORES):
        o = res.results[i]["out"].astype(np.float32)
        # per-phase planar [P, 4, Tl] -> [P, rows, 4]
        parts = []
        oo = 0
        for Tl in TS:
            chunk = o[:, oo * 4:(oo + Tl) * 4].reshape(P, 4, Tl)
            parts.append(chunk.transpose(0, 2, 1))
            oo += Tl
        full = np.concatenate(parts, axis=1).reshape(H, W, 4)
        outs.append(full)
    return np.stack(outs, axis=0), res


def kernel(pixel_colors, zbuf, dists, pix_to_face):
    out, _ = _run(pixel_colors, zbuf, dists, pix_to_face, trace=False)
    return out
